# revision 5
# baseline (speedup 1.0000x reference)
"""Sparse (shot-local + shared-global) attention on 8 Trainium2 NeuronCores.

Problem: B=2, S_TOT=4096, HD=1024 with H=16 heads (d=64), num_shots=4
(L=1024 tokens per shot), global pool = first 64 tokens of each shot
(G=256), shared by all shots of the same batch element.

Sharding: the 32 (batch, head) pairs are split 4-per-core across 8 cores
(data + head parallel). Each (b,h,shot) block is independent attention of
shape q[1024,64] against k/v[1024+256,64].

Key HW facts (measured):
  - PE streams 512-col matmuls at 216ns when the contraction dim is 128
    partitions, but only 427ns when it is 64. So the S^T = k.T @ q
    matmuls (contraction d=64) are zero-padded to K=128: host sends
    q^T/k^T in [128, tokens] tiles with rows 64-127 zeroed. The padding
    rows contribute 0 to the dot products and double the column rate.
  - LDWEIGHTS hides under the matmul stream at this cadence.
  - ACT exp costs ~0.96 ns/psum-column; with 21M logits/core the ACT
    engine (~157us) is the pipeline bottleneck, so everything else
    (PE ~140us, DVE ~22us, DMA ~40us) is arranged to hide beneath it.

Per-core structure: 16 units = (pair, shot); each unit is 10 windows
(8 local k-tiles + 2 global k-tiles); each window w covers one k-tile
against both 512-wide q-chunks:
  S window:  psbig[:, (w%3)*1024 +] = kz_tile.T @ qz (2 matmuls, K=128)
  exp:       expT = exp(psbig_window * 1/8)  (ACT, fused over contiguous
             windows: [128,2048]+[128,1024] per 3 windows)
  PV:        po[qc] += v65_tile.T @ expT (2 matmuls, K=128, 65th row of
             v65 is ones so po row 64 accumulates the softmax denom Z)
  EPI:       DVE copy po -> SBUF, DMA out [65,512] raw (o_unnormalized;Z)
The final softmax division o/Z runs on host during unshard (host already
does the [d,tokens] -> [tokens,d] transpose there).

PSUM: psbig 3 windows x [128,1024] = 6 banks; po pool 4 x [65,512] =
2 banks. Software pipeline: S(w) | exp(w-1 fused) | PV(w-PV_LAG).
"""

import sys

sys.path.insert(0, "/opt/trn_rl_repo")

import ml_dtypes
import numpy as np

import concourse.bass as bass  # noqa: F401  (registers AP machinery)
import concourse.mybir as mybir
import concourse.tile as tile
from concourse import bacc
from concourse.bass_utils import run_bass_kernel_spmd

B, S_TOT, HD = 2, 4096, 1024
H, NSHOT, PER_G = 16, 4, 64
D = HD // H            # 64 head dim
L = S_TOT // NSHOT     # 1024 shot length
G = NSHOT * PER_G      # 256 global pool tokens
NCORES = 8
PAIRS = (B * H) // NCORES   # 4 (b,h) pairs per core
QC = 512                    # q chunk width (PSUM bank)
NQC = L // QC               # 2
NKT_LOC = L // 128          # 8 local k tiles per shot
NKT = NKT_LOC + G // 128    # 10 k tiles (windows) per unit
SCALE = 1.0 / float(np.sqrt(D))
PV_LAG = 4                  # windows between S emission and PV consumption

MM_DT = "float16"
_NC = None


def build_program():
    """Build + compile the per-core Bass program (identical on all cores)."""
    global _NC
    if _NC is not None:
        return _NC
    f32 = mybir.dt.float32
    mdt = getattr(mybir.dt, MM_DT)
    Exp = mybir.ActivationFunctionType.Exp

    nc = bacc.Bacc("TRN2", target_bir_lowering=False, debug=True)
    qz_d = nc.dram_tensor("qz", [128, PAIRS, S_TOT], mdt, kind="ExternalInput")
    kz_d = nc.dram_tensor("kz", [128, PAIRS, S_TOT], mdt, kind="ExternalInput")
    kgz_d = nc.dram_tensor("kgz", [128, PAIRS, G], mdt, kind="ExternalInput")
    v65_d = nc.dram_tensor("v65", [128, PAIRS, NKT_LOC * NSHOT, 65], mdt,
                           kind="ExternalInput")
    vg65_d = nc.dram_tensor("vg65", [128, PAIRS, G // 128, 65], mdt,
                            kind="ExternalInput")
    oZ_d = nc.dram_tensor("oZ", [65, PAIRS, NSHOT * NQC, QC], f32,
                          kind="ExternalOutput")

    with tile.TileContext(nc) as tc:
        with (
            tc.tile_pool(name="inp", bufs=1) as inp_pool,
            tc.tile_pool(name="expp", bufs=1) as exp_pool,
            tc.tile_pool(name="epi", bufs=1) as epi_pool,
            tc.tile_pool(name="ps_s", bufs=1, space="PSUM") as ps_pool,
            tc.tile_pool(name="ps_o", bufs=2, space="PSUM") as po_pool,
        ):
            psbig = ps_pool.tile([128, 3 * 1024], f32, tag="psbig", name="psbig")

            # ---- input loads: all pairs resident; shot-0-of-pair-0 first ----
            sb = []
            for p in range(PAIRS):
                qz = inp_pool.tile([128, S_TOT], mdt, tag=f"qz{p}")
                kz = inp_pool.tile([128, S_TOT], mdt, tag=f"kz{p}")
                kgz = inp_pool.tile([128, G], mdt, tag=f"kgz{p}")
                v65 = inp_pool.tile([128, NKT_LOC * NSHOT, 65], mdt,
                                    tag=f"v65{p}")
                vg65 = inp_pool.tile([128, G // 128, 65], mdt, tag=f"vg65{p}")
                if p == 0:
                    nc.sync.dma_start(qz[:, :L], qz_d[:, p, :L])
                    nc.sync.dma_start(kz[:, :L], kz_d[:, p, :L])
                    nc.sync.dma_start(kgz[:], kgz_d[:, p, :])
                    nc.sync.dma_start(v65[:, :NKT_LOC, :],
                                      v65_d[:, p, :NKT_LOC, :])
                    nc.sync.dma_start(vg65[:], vg65_d[:, p, :, :])
                    nc.sync.dma_start(qz[:, L:], qz_d[:, p, L:])
                    nc.sync.dma_start(kz[:, L:], kz_d[:, p, L:])
                    nc.sync.dma_start(v65[:, NKT_LOC:, :],
                                      v65_d[:, p, NKT_LOC:, :])
                else:
                    nc.sync.dma_start(qz[:], qz_d[:, p, :])
                    nc.sync.dma_start(kz[:], kz_d[:, p, :])
                    nc.sync.dma_start(kgz[:], kgz_d[:, p, :])
                    nc.sync.dma_start(v65[:], v65_d[:, p, :, :])
                    nc.sync.dma_start(vg65[:], vg65_d[:, p, :, :])
                sb.append({"qz": qz, "kz": kz, "kgz": kgz, "v65": v65,
                           "vg65": vg65})

            # ---- window table: 16 units x 10 k-tiles ----
            WINS = []
            for p in range(PAIRS):
                for s in range(NSHOT):
                    for j in range(NKT):
                        WINS.append((p, s, j))
            NW = len(WINS)

            exp_ref = {}   # gw -> (expT tile, col offset)
            po_tiles = {}  # (p, s, qc) -> po tile

            def S_win(gw):
                p, s, j = WINS[gw]
                win = gw % 3
                if j < NKT_LOC:
                    lhsT = sb[p]["kz"][:, s * L + j * 128: s * L + (j + 1) * 128]
                else:
                    gg = j - NKT_LOC
                    lhsT = sb[p]["kgz"][:, gg * 128:(gg + 1) * 128]
                for qc in range(NQC):
                    nc.tensor.matmul(
                        psbig[:, win * 1024 + qc * QC: win * 1024 + (qc + 1) * QC],
                        lhsT,
                        sb[p]["qz"][:, s * L + qc * QC: s * L + (qc + 1) * QC],
                        start=True, stop=True,
                    )

            def emit_exp(g0, g1):
                """One ACT over contiguous psbig windows g0..g1."""
                n = g1 - g0 + 1
                expT = exp_pool.tile([128, 1024 * n], mdt, tag="expT",
                                     name="expT", bufs=6)
                nc.scalar.activation(
                    expT[:], psbig[:, (g0 % 3) * 1024: (g0 % 3 + n) * 1024],
                    Exp, scale=SCALE)
                for i, g in enumerate(range(g0, g1 + 1)):
                    exp_ref[g] = (expT, i * 1024)

            def PV(gw):
                p, s, j = WINS[gw]
                expT, base = exp_ref.pop(gw)
                if j < NKT_LOC:
                    v_lhs = sb[p]["v65"][:, s * NKT_LOC + j, :]
                else:
                    v_lhs = sb[p]["vg65"][:, j - NKT_LOC, :]
                for qc in range(NQC):
                    key = (p, s, qc)
                    if j == 0:
                        po_tiles[key] = po_pool.tile([65, QC], f32, tag="po",
                                                     name="po")
                    nc.tensor.matmul(
                        po_tiles[key][:], v_lhs,
                        expT[:, base + qc * QC: base + (qc + 1) * QC],
                        start=(j == 0), stop=(j == NKT - 1),
                    )
                if j == NKT - 1:
                    for qc in range(NQC):
                        po = po_tiles.pop((p, s, qc))
                        oZ_sb = epi_pool.tile([65, QC], f32, tag="oZ", bufs=4)
                        nc.vector.tensor_copy(oZ_sb[:], po[:])
                        nc.sync.dma_start(
                            oZ_d[:, p, s * NQC + qc, :], oZ_sb[:])

            # ---- software-pipelined emission ----
            # PV first within each step: PV(gw-PV_LAG)'s inputs are ready, so
            # it keeps the in-order Tensor queue streaming while S(gw) may
            # still be blocked on the psbig WAR (exp of gw-3 completing).
            for gw in range(NW + PV_LAG):
                if gw >= PV_LAG:
                    PV(gw - PV_LAG)
                if gw < NW:
                    S_win(gw)
                    if gw % 3 == 1:
                        emit_exp(gw - 1, gw)
                    elif gw % 3 == 2 or gw == NW - 1:
                        emit_exp(gw, gw)
    nc.compile()
    _NC = nc
    return nc


def pack_inputs(q, k, v):
    """Shard + relayout full inputs into per-core input maps."""
    ndt = ml_dtypes.bfloat16 if MM_DT == "bfloat16" else np.float16
    q5 = np.ascontiguousarray(q).reshape(B, S_TOT, H, D)
    k5 = np.ascontiguousarray(k).reshape(B, S_TOT, H, D)
    v5 = np.ascontiguousarray(v).reshape(B, S_TOT, H, D)
    gidx = (np.arange(NSHOT)[:, None] * L + np.arange(PER_G)[None, :]).reshape(-1)

    in_maps = []
    for c in range(NCORES):
        qz = np.zeros((128, PAIRS, S_TOT), ndt)
        kz = np.zeros((128, PAIRS, S_TOT), ndt)
        kgz = np.zeros((128, PAIRS, G), ndt)
        v65 = np.ones((128, PAIRS, NKT_LOC * NSHOT, 65), ndt)
        vg65 = np.ones((128, PAIRS, G // 128, 65), ndt)
        for p in range(PAIRS):
            pair = c * PAIRS + p
            b, h = divmod(pair, H)
            qz[:D, p, :] = q5[b, :, h, :].T
            kz[:D, p, :] = k5[b, :, h, :].T
            kgz[:D, p, :] = k5[b, gidx, h, :].T
            # [S_TOT, 64] -> [n_tiles, 128, 64] -> [128, n_tiles, 64]
            v65[:, p, :, :64] = v5[b, :, h, :].reshape(-1, 128, D).transpose(1, 0, 2)
            vg65[:, p, :, :64] = v5[b, gidx, h, :].reshape(-1, 128, D).transpose(1, 0, 2)
        in_maps.append({"qz": qz, "kz": kz, "kgz": kgz,
                        "v65": v65, "vg65": vg65})
    return in_maps


def unpack_outputs(results):
    """Per-core oZ [65, PAIRS, 8, 512] -> full [B, S_TOT, HD] (softmax
    denominator division happens here on host)."""
    out5 = np.empty((B, S_TOT, H, D), np.float32)
    for c in range(NCORES):
        oZ = results[c]["oZ"]
        o = oZ[:D] / oZ[D:D + 1]
        for p in range(PAIRS):
            b, h = divmod(c * PAIRS + p, H)
            out5[b, :, h, :] = o[:, p].reshape(D, S_TOT).T
    return out5.reshape(B, S_TOT, HD)


def kernel(q, k, v, num_heads, num_shots, per_g):
    assert int(num_heads) == H and int(num_shots) == NSHOT and int(per_g) == PER_G
    nc = build_program()
    in_maps = pack_inputs(np.asarray(q), np.asarray(k), np.asarray(v))
    res = run_bass_kernel_spmd(nc, in_maps, list(range(NCORES)))
    return unpack_outputs(res.results)
